# revision 16
# baseline (speedup 1.0000x reference)
"""GAT (2x GATConv(4 heads,32) + GATConv(1,8) + bn/elu + log_softmax) on 8 trn2 cores.

Strategy: shard destination nodes across 8 cores (6250 each). Per layer each
core computes projected features h=x@W and attention logits a_src/a_dst for its
own nodes, AllGathers a node-feature table [h | a_src] to every core (2 chunks),
then processes its edges in 49 blocks of 128 destination nodes:
  - bulk dma_gather of table rows by edge source id (int16 indices, so the
    table is split in 2 halves per the allgather chunks)
  - per-edge a_dst via: replicate dst_local row (ones-matmul) -> is_equal vs
    iota -> selector matrix selB -> matmul against per-block a_dst columns
  - e = leaky_relu(a_src_g + a_dst_e); ex = exp(e)  (logit range is small, no
    max-subtraction needed; softmax is shift-invariant so result is exact)
  - weighted features ex*h and the ex column aggregated per dst node with a
    single accumulating selector matmul (selA) -> numerator and denominator
  - normalize, bn+elu (fused affine), feed next layer.
Pad edge slots carry dst_local=999 so both selectors are zero there.
"""

import numpy as np

N = 50000
F_IN = 64
HID = 32
HEADS = 4
NCLS = 8
BN_EPS = 1e-5
NEG_SLOPE = 0.2
NCORES = 8

NLOC = N // NCORES          # 6250
NBLK = (NLOC + 127) // 128  # 49
NLOCP = NBLK * 128          # 6272
HALF0 = 3200                # allgather chunk A rows per core
HALF1 = NLOCP - HALF0       # 3072
TW = 256                    # bf16 table row width, layers 1-2 (512B)
TW3 = 128                   # bf16 layer-3 table row width (256B)
PADLOC = 999.0              # dst_local sentinel for pad slots


def _fold_params(params):
    """Host-side folding of the small parameter tensors (O(params) work only)."""
    f32 = np.float32
    w = {}

    def bn(p):
        s = np.asarray(p["gamma"]) / np.sqrt(np.asarray(p["var"]) + BN_EPS)
        b = np.asarray(p["beta"]) - np.asarray(p["mean"]) * s
        return s.astype(f32), b.astype(f32)

    bnin_s, bnin_b = bn(params["bn_input"])
    w["bnin_s"] = bnin_s[:, None]
    w["bnin_b"] = bnin_b[:, None]
    w["Wproj"] = np.asarray(params["proj_W"], f32)
    bnp_s, bnp_b = bn(params["bn_proj"])
    w["bnp_s"] = bnp_s[:, None]
    w["bnp_b"] = (np.asarray(params["proj_b"], f32) * bnp_s + bnp_b)[:, None]

    def att_mat(a, cout):
        # a: [H, C] -> block diagonal [H*C, H]
        a = np.asarray(a, f32)
        H, C = a.shape
        m = np.zeros((H * C, H), f32)
        for h in range(H):
            m[h * C:(h + 1) * C, h] = a[h]
        return m

    for i, (cp, bp) in enumerate(zip(params["convs"], params["bns"]), start=1):
        W = np.asarray(cp["W"], f32)
        w[f"W{i}"] = W
        w[f"c{i}"] = W.sum(axis=0).astype(f32)[:, None]   # W^T @ 1
        w[f"AS{i}"] = att_mat(cp["att_src"], HID)
        w[f"AD{i}"] = att_mat(cp["att_dst"], HID)
        s, b = bn(bp)
        w[f"bn{i}_s"] = s[:, None]
        w[f"bn{i}_b"] = (np.asarray(cp["bias"], f32) * s + b)[:, None]
    cp = params["conv_out"]
    W = np.asarray(cp["W"], f32)
    w["W3"] = W
    w["c3"] = W.sum(axis=0).astype(f32)[:, None]
    w["AS3"] = np.asarray(cp["att_src"], f32).reshape(NCLS, 1)
    w["AD3"] = np.asarray(cp["att_dst"], f32).reshape(NCLS, 1)
    w["bias3_rep"] = np.broadcast_to(np.asarray(cp["bias"], f32)[None, :], (128, NCLS)).copy()

    import ml_dtypes
    w["iota_row"] = np.broadcast_to(
        np.arange(128, dtype=ml_dtypes.bfloat16)[None, :], (128, 128)).copy()
    w["iota_col"] = np.arange(128, dtype=f32)[:, None].copy()
    w["ones_row"] = np.ones((1, 128), f32)
    w["ident"] = np.eye(128, dtype=f32)
    return w


def _preprocess_edges(edge_index):
    """Bucket edges (incl self loops) by (dst core, dst block, src half).

    Returns per-core index/dst_local/flat arrays plus the global pad size CH.
    """
    src0 = np.asarray(edge_index[0], np.int64)
    dst0 = np.asarray(edge_index[1], np.int64)
    loops = np.arange(N, dtype=np.int64)
    src = np.concatenate([src0, loops])
    dst = np.concatenate([dst0, loops])

    core = dst // NLOC
    dl = dst % NLOC
    blk = dl // 128
    p = dl % 128
    scre = src // NLOC
    srcn = src % NLOC
    g = scre * NLOCP + srcn  # core-major row in the allgathered table
    halfb = g >= (NCORES // 2) * NLOCP
    # gather index within the contiguous half-view of the full table
    idx = np.where(halfb, g - (NCORES // 2) * NLOCP, g)

    key = ((core * NBLK + blk) * 2 + halfb.astype(np.int64))
    order = np.argsort(key, kind="stable")
    key_s = key[order]
    cnt = np.bincount(key_s, minlength=NCORES * NBLK * 2)
    CH = int(-(-cnt.max() // 128) * 128)
    NT = 2 * CH // 128
    starts = np.zeros_like(cnt)
    starts[1:] = np.cumsum(cnt)[:-1]
    seq = np.arange(len(key_s)) - starts[key_s]

    idx_s = idx[order]
    p_s = p[order]
    core_s = key_s // (NBLK * 2)
    blk_s = (key_s // 2) % NBLK
    half_s = key_s % 2

    IC = CH // 16  # idx cols per (block, half)
    per_core = []
    for r in range(NCORES):
        m = core_s == r
        b_r = blk_s[m]
        h_r = half_s[m]
        j_r = seq[m]
        idx_r = idx_s[m]
        p_r = p_s[m]

        idx16 = np.zeros((16, NBLK * 2 * IC), np.int16)
        idx16[j_r % 16, (b_r * 2 + h_r) * IC + j_r // 16] = idx_r.astype(np.int16)
        idx16 = np.tile(idx16, (8, 1))  # replicate across the 8 q7 core groups

        s_r = h_r * CH + j_r  # slot within block
        import ml_dtypes
        dstloc = np.full((128, NBLK * NT), PADLOC, ml_dtypes.bfloat16)
        dstloc[s_r % 128, b_r * NT + s_r // 128] = p_r.astype(ml_dtypes.bfloat16)
        dstflat = np.full((NBLK, 2 * CH), PADLOC, np.float32)
        dstflat[b_r, s_r] = p_r.astype(np.float32)
        per_core.append(dict(idx16=idx16, dstloc=dstloc, dstflat=dstflat))
    return per_core, CH


def _build_nc(CH, single=False):
    import concourse.bacc as bacc
    import concourse.bass as bass
    import concourse.tile as tile
    from concourse import mybir

    f32 = mybir.dt.float32
    bf16 = mybir.dt.bfloat16
    NT = 2 * CH // 128
    CHT = CH // 128
    IC = CH // 16
    EBLK = 2 * CH
    ALU = mybir.AluOpType
    AF = mybir.ActivationFunctionType

    nc = bacc.Bacc("TRN2", target_bir_lowering=False, debug=False,
                   num_devices=1 if single else NCORES)

    # ---- dram I/O ----
    din = {}
    def inp(name, shape, dtype=f32):
        din[name] = nc.dram_tensor(name, list(shape), dtype, kind="ExternalInput")
        return din[name]

    inp("xT", (F_IN, NLOCP))
    inp("idx16", (128, NBLK * 2 * IC), mybir.dt.int16)
    inp("dstloc", (128, NBLK * NT), bf16)
    inp("dstflat", (NBLK, EBLK))
    wnames = [
        ("bnin_s", (F_IN, 1)), ("bnin_b", (F_IN, 1)), ("Wproj", (F_IN, HID)),
        ("bnp_s", (HID, 1)), ("bnp_b", (HID, 1)),
        ("W1", (HID, 128)), ("c1", (128, 1)), ("AS1", (128, 4)), ("AD1", (128, 4)),
        ("bn1_s", (128, 1)), ("bn1_b", (128, 1)),
        ("W2", (128, 128)), ("c2", (128, 1)), ("AS2", (128, 4)), ("AD2", (128, 4)),
        ("bn2_s", (128, 1)), ("bn2_b", (128, 1)),
        ("W3", (128, NCLS)), ("c3", (NCLS, 1)), ("AS3", (NCLS, 1)), ("AD3", (NCLS, 1)),
        ("bias3_rep", (128, NCLS)),
        ("iota_col", (128, 1)), ("ones_row", (1, 128)),
        ("ident", (128, 128)),
    ]
    for nm, shp in wnames:
        inp(nm, shp)
    inp("iota_row", (128, 128), bf16)
    out_dram = nc.dram_tensor("out", [NLOCP, NCLS], f32, kind="ExternalOutput")

    slices = {}
    tabs = {}
    for l, tw in ((1, TW), (2, TW), (3, TW3)):
        slices[l] = nc.dram_tensor(f"slice{l}", [NLOCP, tw], bf16)
        tabs[l] = nc.dram_tensor(f"tab{l}", [NCORES * NLOCP, tw], bf16, addr_space="Shared")

    LAYERS = {
        1: dict(Cin=HID, Cout=128, H=4, tw=TW, W="W1", C="c1", AS="AS1", AD="AD1"),
        2: dict(Cin=128, Cout=128, H=4, tw=TW, W="W2", C="c2", AS="AS2", AD="AD2"),
        3: dict(Cin=128, Cout=NCLS, H=1, tw=TW3, W="W3", C="c3", AS="AS3", AD="AD3"),
    }

    chunks = []
    off = 0
    while off < NLOCP:
        sz = min(512, NLOCP - off)
        chunks.append((off, sz))
        off += sz

    with tile.TileContext(nc) as tc:
        with (
            tc.tile_pool(name="const", bufs=1) as cpool,
            tc.tile_pool(name="persist", bufs=1) as ppool,
            tc.tile_pool(name="work", bufs=2) as wpool,
            tc.tile_pool(name="ps2", bufs=2, space="PSUM") as ps2,
            tc.tile_pool(name="ps1", bufs=1, space="PSUM") as ps1,
        ):
            # ---- load constants ----
            W = {}
            for nm, shp in wnames:
                t = cpool.tile(list(shp), f32, tag=f"w_{nm}")
                nc.sync.dma_start(out=t[:], in_=din[nm].ap())
                W[nm] = t
            t_ir = cpool.tile([128, 128], bf16, tag="w_iota_row", name="iota_row_sb")
            nc.sync.dma_start(out=t_ir[:], in_=din["iota_row"].ap())
            W["iota_row"] = t_ir
            idx_sb = ppool.tile([128, NBLK * 2 * IC], mybir.dt.int16, tag="idx16")
            nc.sync.dma_start(out=idx_sb[:], in_=din["idx16"].ap())
            dstloc_sb = ppool.tile([128, NBLK * NT], bf16, tag="dstloc")
            nc.sync.dma_start(out=dstloc_sb[:], in_=din["dstloc"].ap())

            x1T = ppool.tile([HID, NLOCP], f32, tag="x1T")
            xbig = {2: ppool.tile([128, NLOCP], f32, tag="xbig", name="x2T"),
                    3: ppool.tile([128, NLOCP], f32, tag="xbig", name="x3T")}
            adst_nm = ppool.tile([128, NBLK * 4], bf16, tag="adstnm")

            # ---- prep: bn_input -> proj -> bn_proj -> elu(+1) ----
            with tc.tile_pool(name="prep", bufs=2) as qpool:
                for off, sz in chunks:
                    xc = qpool.tile([F_IN, 512], f32, tag="xc")
                    nc.sync.dma_start(out=xc[:, :sz], in_=din["xT"].ap()[:, off:off + sz])
                    xb = qpool.tile([F_IN, 512], f32, tag="xb")
                    nc.scalar.activation(out=xb[:, :sz], in_=xc[:, :sz],
                                         func=AF.Identity, scale=W["bnin_s"][:],
                                         bias=W["bnin_b"][:])
                    ph = ps2.tile([HID, 512], f32, tag="mm")
                    nc.tensor.matmul(out=ph[:, :sz], lhsT=W["Wproj"][:], rhs=xb[:, :sz],
                                     start=True, stop=True)
                    y = qpool.tile([HID, 512], f32, tag="y32")
                    nc.vector.tensor_scalar(out=y[:, :sz], in0=ph[:, :sz],
                                            scalar1=W["bnp_s"][:], scalar2=W["bnp_b"][:],
                                            op0=ALU.mult, op1=ALU.add)
                    ng = qpool.tile([HID, 512], f32, tag="ng32")
                    nc.vector.tensor_scalar(out=ng[:, :sz], in0=y[:, :sz], scalar1=0.0,
                                            scalar2=None, op0=ALU.min)
                    en = qpool.tile([HID, 512], f32, tag="en32")
                    nc.scalar.activation(out=en[:, :sz], in_=ng[:, :sz], func=AF.Exp)
                    nc.vector.scalar_tensor_tensor(out=x1T[:, off:off + sz], in0=y[:, :sz],
                                                   scalar=0.0, in1=en[:, :sz],
                                                   op0=ALU.max, op1=ALU.add)

            # ---- per layer ----
            for l in (1, 2, 3):
                P = LAYERS[l]
                Cout, H, tw = P["Cout"], P["H"], P["tw"]
                xcur = x1T if l == 1 else xbig[l]
                # dense + table build
                with tc.tile_pool(name=f"dense{l}", bufs=2) as dpool:
                    for off, sz in chunks:
                        ph = ps2.tile([Cout, 512], f32, tag="mm")
                        nc.tensor.matmul(out=ph[:, :sz], lhsT=W[P["W"]][:],
                                         rhs=xcur[:, off:off + sz], start=True, stop=True)
                        hTc = dpool.tile([Cout, 512], f32, tag="hTc")
                        nc.vector.tensor_scalar(out=hTc[:, :sz], in0=ph[:, :sz],
                                                scalar1=W[P["C"]][:], scalar2=None,
                                                op0=ALU.subtract)
                        pas = ps2.tile([H, 512], f32, tag="aux")
                        nc.tensor.matmul(out=pas[:, :sz], lhsT=W[P["AS"]][:],
                                         rhs=hTc[:, :sz], start=True, stop=True)
                        asr = dpool.tile([H, 512], f32, tag="asr")
                        nc.vector.tensor_copy(out=asr[:, :sz], in_=pas[:, :sz])
                        pad_ = ps2.tile([H, 512], f32, tag="aux")
                        nc.tensor.matmul(out=pad_[:, :sz], lhsT=W[P["AD"]][:],
                                         rhs=hTc[:, :sz], start=True, stop=True)
                        adr = dpool.tile([H, 512], f32, tag="adr")
                        nc.vector.tensor_copy(out=adr[:, :sz], in_=pad_[:, :sz])
                        for nt in range(sz // 128):
                            g = off // 128 + nt  # global node tile == block id
                            lo = nt * 128
                            pt = ps2.tile([128, Cout], f32, tag="aux")
                            nc.tensor.transpose(out=pt[:], in_=hTc[:, lo:lo + 128],
                                                identity=W["ident"][:Cout, :Cout])
                            stg = dpool.tile([128, tw], bf16, tag="stg")
                            nc.vector.tensor_copy(out=stg[:, 0:Cout], in_=pt[:])
                            pt2 = ps2.tile([128, H], f32, tag="aux")
                            nc.tensor.transpose(out=pt2[:], in_=asr[:, lo:lo + 128],
                                                identity=W["ident"][:H, :H])
                            nc.vector.tensor_copy(out=stg[:, Cout:Cout + H], in_=pt2[:])
                            pt3 = ps2.tile([128, H], f32, tag="aux")
                            nc.tensor.transpose(out=pt3[:], in_=adr[:, lo:lo + 128],
                                                identity=W["ident"][:H, :H])
                            nc.vector.tensor_copy(out=adst_nm[:, g * 4:g * 4 + H], in_=pt3[:])
                            nc.sync.dma_start(
                                out=slices[l].ap()[g * 128:(g + 1) * 128, 0:Cout + H],
                                in_=stg[:, 0:Cout + H])
                # allgather the table (one collective per layer)
                if single:
                    # timing stand-in: move the same bytes via plain DMA
                    for r in range(NCORES):
                        nc.sync.dma_start(
                            out=tabs[l].ap()[r * NLOCP:(r + 1) * NLOCP, :],
                            in_=slices[l].ap()[:, :])
                else:
                    nc.gpsimd.collective_compute(
                        "AllGather", ALU.bypass,
                        replica_groups=[list(range(NCORES))],
                        ins=[slices[l].ap()[:, :]],
                        outs=[tabs[l].ap()],
                    )

                # ---- edge phase ----
                ac = Cout          # a_src column base in gathered rows
                ec = Cout + H      # ex column base
                AW = Cout + 2 * H  # aggregated width
                for b in range(NBLK):
                    Z = wpool.tile([128, NT, tw], bf16, tag="Z")
                    HROWS = (NCORES // 2) * NLOCP
                    for h in range(2):
                        nc.gpsimd.dma_gather(
                            out_ap=Z[:, h * CHT:(h + 1) * CHT, :],
                            in_ap=tabs[l].ap()[h * HROWS:(h + 1) * HROWS, :],
                            idxs_ap=idx_sb[:, (b * 2 + h) * IC:(b * 2 + h + 1) * IC],
                            num_idxs=CH, num_idxs_reg=CH, elem_size=tw,
                            single_packet=False)
                    selA = wpool.tile([128, NT, 128], bf16, tag="selA")
                    nc.vector.tensor_tensor(
                        out=selA[:],
                        in0=dstloc_sb[:, b * NT:(b + 1) * NT][:, :, None].to_broadcast([128, NT, 128]),
                        in1=W["iota_row"][:, None, :].to_broadcast([128, NT, 128]),
                        op=ALU.is_equal)
                    selB = wpool.tile([128, EBLK], bf16, tag="selB")
                    for c5 in range(EBLK // 512):
                        dfp0 = wpool.tile([1, 512], f32, tag="dfp0")
                        nc.sync.dma_start(
                            out=dfp0[:],
                            in_=din["dstflat"].ap()[b:b + 1, c5 * 512:(c5 + 1) * 512])
                        pr = ps2.tile([128, 512], f32, tag="mm")
                        nc.tensor.matmul(out=pr[:], lhsT=W["ones_row"][:],
                                         rhs=dfp0[:], start=True, stop=True)
                        nc.vector.tensor_tensor(
                            out=selB[:, c5 * 512:(c5 + 1) * 512],
                            in0=W["iota_col"][:].to_broadcast([128, 512]),
                            in1=pr[:], op=ALU.is_equal)
                    pad_ps = ps1.tile([128, NT, H], f32, tag="adst")
                    for k in range(NT):
                        nc.tensor.matmul(out=pad_ps[:, k, :],
                                         lhsT=selB[:, k * 128:(k + 1) * 128],
                                         rhs=adst_nm[:, b * 4:b * 4 + H],
                                         start=True, stop=True)
                    s_e = wpool.tile([128, NT, H], f32, tag="se")
                    nc.vector.tensor_tensor(out=s_e[:], in0=Z[:, :, ac:ac + H],
                                            in1=pad_ps[:], op=ALU.add)
                    nc.vector.scalar_tensor_tensor(out=s_e[:], in0=s_e[:],
                                                   scalar=NEG_SLOPE, in1=s_e[:],
                                                   op0=ALU.mult, op1=ALU.max)
                    nc.scalar.activation(out=Z[:, :, ec:ec + H], in_=s_e[:], func=AF.Exp)
                    zv = Z[:, :, 0:Cout].rearrange("p k (h c) -> p k h c", h=H)
                    exb = Z[:, :, ec:ec + H][:, :, :, None].to_broadcast(
                        [128, NT, H, Cout // H])
                    nc.vector.tensor_tensor(out=zv, in0=zv, in1=exb, op=ALU.mult)
                    pagg = ps2.tile([128, AW], f32, tag="agg")
                    for k in range(NT):
                        nc.tensor.matmul(out=pagg[:], lhsT=selA[:, k, :],
                                         rhs=Z[:, k, 0:AW],
                                         start=(k == 0), stop=(k == NT - 1))
                    r4 = wpool.tile([128, H], f32, tag="r4")
                    nc.vector.tensor_scalar(out=r4[:], in0=pagg[:, AW - H:AW],
                                            scalar1=1e-16, scalar2=None, op0=ALU.add)
                    nc.vector.reciprocal(out=r4[:], in_=r4[:])
                    o_sb = wpool.tile([128, Cout], f32, tag="osb")
                    nc.vector.tensor_tensor(
                        out=o_sb[:].rearrange("p (h c) -> p h c", h=H),
                        in0=pagg[:, 0:Cout].rearrange("p (h c) -> p h c", h=H),
                        in1=r4[:][:, :, None].to_broadcast([128, H, Cout // H]),
                        op=ALU.mult)
                    if l < 3:
                        pt = ps2.tile([128, 128], f32, tag="aux")
                        nc.tensor.transpose(out=pt[:], in_=o_sb[:], identity=W["ident"][:])
                        y = wpool.tile([128, 128], f32, tag="yh")
                        nc.vector.tensor_scalar(out=y[:], in0=pt[:],
                                                scalar1=W[f"bn{l}_s"][:],
                                                scalar2=W[f"bn{l}_b"][:],
                                                op0=ALU.mult, op1=ALU.add)
                        ng = wpool.tile([128, 128], f32, tag="ngh")
                        nc.vector.tensor_scalar(out=ng[:], in0=y[:], scalar1=0.0,
                                                scalar2=None, op0=ALU.min)
                        en = wpool.tile([128, 128], f32, tag="enh")
                        nc.scalar.activation(out=en[:], in_=ng[:], func=AF.Exp)
                        nc.vector.scalar_tensor_tensor(
                            out=xbig[l + 1][:, b * 128:(b + 1) * 128], in0=y[:],
                            scalar=0.0, in1=en[:], op0=ALU.max, op1=ALU.add)
                    else:
                        nc.vector.tensor_tensor(out=o_sb[:], in0=o_sb[:],
                                                in1=W["bias3_rep"][:], op=ALU.add)
                        mx = wpool.tile([128, 1], f32, tag="mx")
                        nc.vector.tensor_reduce(out=mx[:], in_=o_sb[:],
                                                axis=mybir.AxisListType.X, op=ALU.max)
                        sh = wpool.tile([128, NCLS], f32, tag="sh")
                        nc.vector.tensor_scalar(out=sh[:], in0=o_sb[:], scalar1=mx[:],
                                                scalar2=None, op0=ALU.subtract)
                        ex8 = wpool.tile([128, NCLS], f32, tag="ex8")
                        se = wpool.tile([128, 1], f32, tag="sse")
                        nc.scalar.activation(out=ex8[:], in_=sh[:], func=AF.Exp,
                                             accum_out=se[:])
                        lnse = wpool.tile([128, 1], f32, tag="lnse")
                        nc.scalar.activation(out=lnse[:], in_=se[:], func=AF.Ln)
                        res = wpool.tile([128, NCLS], f32, tag="res")
                        nc.vector.tensor_scalar(out=res[:], in0=sh[:], scalar1=lnse[:],
                                                scalar2=None, op0=ALU.subtract)
                        nc.sync.dma_start(out=out_dram.ap()[b * 128:(b + 1) * 128, :],
                                          in_=res[:])

    nc.compile()
    return nc


def kernel(x, edge_index, params):
    x = np.asarray(x, np.float32)
    w = _fold_params(params)
    per_core, CH = _preprocess_edges(edge_index)

    nc = _build_nc(CH)

    in_maps = []
    for r in range(NCORES):
        m = dict(w)
        xr = np.zeros((NLOCP, F_IN), np.float32)
        xr[:NLOC] = x[r * NLOC:(r + 1) * NLOC]
        m["xT"] = np.ascontiguousarray(xr.T)
        m["idx16"] = per_core[r]["idx16"]
        m["dstloc"] = per_core[r]["dstloc"]
        m["dstflat"] = per_core[r]["dstflat"]
        in_maps.append(m)

    from concourse.bass_utils import run_bass_kernel_spmd
    import os
    trace = os.environ.get("GAT_TRACE", "0") == "1"
    res = run_bass_kernel_spmd(nc, in_maps, core_ids=list(range(NCORES)), trace=trace)
    if trace:
        kernel.last_results = res
        print("HW exec time:", res.exec_time_ns, "ns")
    out = np.concatenate([res.results[r]["out"][:NLOC] for r in range(NCORES)], axis=0)
    return out


# revision 18
# speedup vs baseline: 1.1607x; 1.1607x over previous
"""GAT (2x GATConv(4 heads,32) + GATConv(1,8) + bn/elu + log_softmax) on 8 trn2 cores.

Strategy: shard destination nodes across 8 cores (6250 each). Per layer each
core computes projected features h=x@W and attention logits a_src/a_dst for its
own nodes, AllGathers a node-feature table [h | a_src] to every core (2 chunks),
then processes its edges in 49 blocks of 128 destination nodes:
  - bulk dma_gather of table rows by edge source id (int16 indices, so the
    table is split in 2 halves per the allgather chunks)
  - per-edge a_dst via: replicate dst_local row (ones-matmul) -> is_equal vs
    iota -> selector matrix selB -> matmul against per-block a_dst columns
  - e = leaky_relu(a_src_g + a_dst_e); ex = exp(e)  (logit range is small, no
    max-subtraction needed; softmax is shift-invariant so result is exact)
  - weighted features ex*h and the ex column aggregated per dst node with a
    single accumulating selector matmul (selA) -> numerator and denominator
  - normalize, bn+elu (fused affine), feed next layer.
Pad edge slots carry dst_local=999 so both selectors are zero there.
"""

import numpy as np

N = 50000
F_IN = 64
HID = 32
HEADS = 4
NCLS = 8
BN_EPS = 1e-5
NEG_SLOPE = 0.2
NCORES = 8

NLOC = N // NCORES          # 6250
NBLK = (NLOC + 127) // 128  # 49
NLOCP = NBLK * 128          # 6272
HALF0 = 3200                # allgather chunk A rows per core
HALF1 = NLOCP - HALF0       # 3072
TW = 256                    # bf16 table row width, layers 1-2 (512B)
TW3 = 128                   # bf16 layer-3 table row width (256B)
PADLOC = 999.0              # dst_local sentinel for pad slots


def _fold_params(params):
    """Host-side folding of the small parameter tensors (O(params) work only)."""
    f32 = np.float32
    w = {}

    def bn(p):
        s = np.asarray(p["gamma"]) / np.sqrt(np.asarray(p["var"]) + BN_EPS)
        b = np.asarray(p["beta"]) - np.asarray(p["mean"]) * s
        return s.astype(f32), b.astype(f32)

    bnin_s, bnin_b = bn(params["bn_input"])
    w["bnin_s"] = bnin_s[:, None]
    w["bnin_b"] = bnin_b[:, None]
    w["Wproj"] = np.asarray(params["proj_W"], f32)
    bnp_s, bnp_b = bn(params["bn_proj"])
    w["bnp_s"] = bnp_s[:, None]
    w["bnp_b"] = (np.asarray(params["proj_b"], f32) * bnp_s + bnp_b)[:, None]

    def att_mat(a, cout):
        # a: [H, C] -> block diagonal [H*C, H]
        a = np.asarray(a, f32)
        H, C = a.shape
        m = np.zeros((H * C, H), f32)
        for h in range(H):
            m[h * C:(h + 1) * C, h] = a[h]
        return m

    for i, (cp, bp) in enumerate(zip(params["convs"], params["bns"]), start=1):
        W = np.asarray(cp["W"], f32)
        w[f"W{i}"] = W
        w[f"c{i}"] = W.sum(axis=0).astype(f32)[:, None]   # W^T @ 1
        w[f"AS{i}"] = att_mat(cp["att_src"], HID)
        w[f"AD{i}"] = att_mat(cp["att_dst"], HID)
        s, b = bn(bp)
        w[f"bn{i}_s"] = s[:, None]
        w[f"bn{i}_b"] = (np.asarray(cp["bias"], f32) * s + b)[:, None]
    cp = params["conv_out"]
    W = np.asarray(cp["W"], f32)
    w["W3"] = W
    w["c3"] = W.sum(axis=0).astype(f32)[:, None]
    w["AS3"] = np.asarray(cp["att_src"], f32).reshape(NCLS, 1)
    w["AD3"] = np.asarray(cp["att_dst"], f32).reshape(NCLS, 1)
    w["bias3_rep"] = np.broadcast_to(np.asarray(cp["bias"], f32)[None, :], (128, NCLS)).copy()

    import ml_dtypes
    w["iota_row"] = np.broadcast_to(
        np.arange(128, dtype=ml_dtypes.bfloat16)[None, :], (128, 128)).copy()
    w["iota_col"] = np.arange(128, dtype=f32)[:, None].copy()
    w["ones_row"] = np.ones((1, 128), f32)
    w["ident"] = np.eye(128, dtype=f32)
    return w


def _preprocess_edges(edge_index):
    """Bucket edges (incl self loops) by (dst core, dst block, src half).

    Returns per-core index/dst_local/flat arrays plus the global pad size CH.
    """
    src0 = np.asarray(edge_index[0], np.int64)
    dst0 = np.asarray(edge_index[1], np.int64)
    loops = np.arange(N, dtype=np.int64)
    src = np.concatenate([src0, loops])
    dst = np.concatenate([dst0, loops])

    core = dst // NLOC
    dl = dst % NLOC
    blk = dl // 128
    p = dl % 128
    scre = src // NLOC
    srcn = src % NLOC
    g = scre * NLOCP + srcn  # core-major row in the allgathered table
    halfb = g >= (NCORES // 2) * NLOCP
    # gather index within the contiguous half-view of the full table
    idx = np.where(halfb, g - (NCORES // 2) * NLOCP, g)

    key = ((core * NBLK + blk) * 2 + halfb.astype(np.int64))
    order = np.argsort(key, kind="stable")
    key_s = key[order]
    cnt = np.bincount(key_s, minlength=NCORES * NBLK * 2)
    CH = int(-(-cnt.max() // 128) * 128)
    NT = 2 * CH // 128
    starts = np.zeros_like(cnt)
    starts[1:] = np.cumsum(cnt)[:-1]
    seq = np.arange(len(key_s)) - starts[key_s]

    idx_s = idx[order]
    p_s = p[order]
    core_s = key_s // (NBLK * 2)
    blk_s = (key_s // 2) % NBLK
    half_s = key_s % 2

    IC = CH // 16  # idx cols per (block, half)
    per_core = []
    for r in range(NCORES):
        m = core_s == r
        b_r = blk_s[m]
        h_r = half_s[m]
        j_r = seq[m]
        idx_r = idx_s[m]
        p_r = p_s[m]

        idx16 = np.zeros((16, NBLK * 2 * IC), np.int16)
        idx16[j_r % 16, (b_r * 2 + h_r) * IC + j_r // 16] = idx_r.astype(np.int16)
        idx16 = np.tile(idx16, (8, 1))  # replicate across the 8 q7 core groups

        s_r = h_r * CH + j_r  # slot within block
        import ml_dtypes
        dstloc = np.full((128, NBLK * NT), PADLOC, ml_dtypes.bfloat16)
        dstloc[s_r % 128, b_r * NT + s_r // 128] = p_r.astype(ml_dtypes.bfloat16)
        dstflat = np.full((NBLK, 2 * CH), PADLOC, np.float32)
        dstflat[b_r, s_r] = p_r.astype(np.float32)
        per_core.append(dict(idx16=idx16, dstloc=dstloc, dstflat=dstflat))
    return per_core, CH


def _build_nc(CH, single=False):
    import concourse.bacc as bacc
    import concourse.bass as bass
    import concourse.tile as tile
    from concourse import mybir

    f32 = mybir.dt.float32
    bf16 = mybir.dt.bfloat16
    NT = 2 * CH // 128
    CHT = CH // 128
    IC = CH // 16
    EBLK = 2 * CH
    ALU = mybir.AluOpType
    AF = mybir.ActivationFunctionType

    nc = bacc.Bacc("TRN2", target_bir_lowering=False, debug=False,
                   num_devices=1 if single else NCORES)

    # ---- dram I/O ----
    din = {}
    def inp(name, shape, dtype=f32):
        din[name] = nc.dram_tensor(name, list(shape), dtype, kind="ExternalInput")
        return din[name]

    inp("xT", (F_IN, NLOCP))
    inp("idx16", (128, NBLK * 2 * IC), mybir.dt.int16)
    inp("dstloc", (128, NBLK * NT), bf16)
    inp("dstflat", (NBLK, EBLK))
    wnames = [
        ("bnin_s", (F_IN, 1)), ("bnin_b", (F_IN, 1)), ("Wproj", (F_IN, HID)),
        ("bnp_s", (HID, 1)), ("bnp_b", (HID, 1)),
        ("W1", (HID, 128)), ("c1", (128, 1)), ("AS1", (128, 4)), ("AD1", (128, 4)),
        ("bn1_s", (128, 1)), ("bn1_b", (128, 1)),
        ("W2", (128, 128)), ("c2", (128, 1)), ("AS2", (128, 4)), ("AD2", (128, 4)),
        ("bn2_s", (128, 1)), ("bn2_b", (128, 1)),
        ("W3", (128, NCLS)), ("c3", (NCLS, 1)), ("AS3", (NCLS, 1)), ("AD3", (NCLS, 1)),
        ("bias3_rep", (128, NCLS)),
        ("iota_col", (128, 1)), ("ones_row", (1, 128)),
        ("ident", (128, 128)),
    ]
    for nm, shp in wnames:
        inp(nm, shp)
    inp("iota_row", (128, 128), bf16)
    out_dram = nc.dram_tensor("out", [NLOCP, NCLS], f32, kind="ExternalOutput")

    slices = {}
    tabs = {}
    for l, tw in ((1, TW), (2, TW), (3, TW3)):
        slices[l] = nc.dram_tensor(f"slice{l}", [NLOCP, tw], bf16)
        tabs[l] = nc.dram_tensor(f"tab{l}", [NCORES * NLOCP, tw], bf16, addr_space="Shared")

    LAYERS = {
        1: dict(Cin=HID, Cout=128, H=4, tw=TW, W="W1", C="c1", AS="AS1", AD="AD1"),
        2: dict(Cin=128, Cout=128, H=4, tw=TW, W="W2", C="c2", AS="AS2", AD="AD2"),
        3: dict(Cin=128, Cout=NCLS, H=1, tw=TW3, W="W3", C="c3", AS="AS3", AD="AD3"),
    }

    chunks = []
    off = 0
    while off < NLOCP:
        sz = min(512, NLOCP - off)
        chunks.append((off, sz))
        off += sz

    with tile.TileContext(nc) as tc:
        with (
            tc.tile_pool(name="const", bufs=1) as cpool,
            tc.tile_pool(name="persist", bufs=1) as ppool,
            tc.tile_pool(name="work", bufs=2) as wpool,
            tc.tile_pool(name="ps2", bufs=2, space="PSUM") as ps2,
            tc.tile_pool(name="ps1", bufs=1, space="PSUM") as ps1,
        ):
            # ---- load constants ----
            W = {}
            for nm, shp in wnames:
                t = cpool.tile(list(shp), f32, tag=f"w_{nm}")
                nc.sync.dma_start(out=t[:], in_=din[nm].ap())
                W[nm] = t
            t_ir = cpool.tile([128, 128], bf16, tag="w_iota_row", name="iota_row_sb")
            nc.sync.dma_start(out=t_ir[:], in_=din["iota_row"].ap())
            W["iota_row"] = t_ir
            idx_sb = ppool.tile([128, NBLK * 2 * IC], mybir.dt.int16, tag="idx16")
            nc.sync.dma_start(out=idx_sb[:], in_=din["idx16"].ap())
            dstloc_sb = ppool.tile([128, NBLK * NT], bf16, tag="dstloc")
            nc.sync.dma_start(out=dstloc_sb[:], in_=din["dstloc"].ap())

            x1T = ppool.tile([HID, NLOCP], f32, tag="x1T")
            xbig = {2: ppool.tile([128, NLOCP], f32, tag="xbig", name="x2T"),
                    3: ppool.tile([128, NLOCP], f32, tag="xbig", name="x3T")}
            adst_nm = ppool.tile([128, NBLK * 4], bf16, tag="adstnm")

            # ---- prep: bn_input -> proj -> bn_proj -> elu(+1) ----
            with tc.tile_pool(name="prep", bufs=2) as qpool:
                for off, sz in chunks:
                    xc = qpool.tile([F_IN, 512], f32, tag="xc")
                    nc.sync.dma_start(out=xc[:, :sz], in_=din["xT"].ap()[:, off:off + sz])
                    xb = qpool.tile([F_IN, 512], f32, tag="xb")
                    nc.scalar.activation(out=xb[:, :sz], in_=xc[:, :sz],
                                         func=AF.Identity, scale=W["bnin_s"][:],
                                         bias=W["bnin_b"][:])
                    ph = ps2.tile([HID, 512], f32, tag="mm")
                    nc.tensor.matmul(out=ph[:, :sz], lhsT=W["Wproj"][:], rhs=xb[:, :sz],
                                     start=True, stop=True)
                    y = qpool.tile([HID, 512], f32, tag="y32")
                    nc.vector.tensor_scalar(out=y[:, :sz], in0=ph[:, :sz],
                                            scalar1=W["bnp_s"][:], scalar2=W["bnp_b"][:],
                                            op0=ALU.mult, op1=ALU.add)
                    ng = qpool.tile([HID, 512], f32, tag="ng32")
                    nc.vector.tensor_scalar(out=ng[:, :sz], in0=y[:, :sz], scalar1=0.0,
                                            scalar2=None, op0=ALU.min)
                    en = qpool.tile([HID, 512], f32, tag="en32")
                    nc.scalar.activation(out=en[:, :sz], in_=ng[:, :sz], func=AF.Exp)
                    nc.vector.scalar_tensor_tensor(out=x1T[:, off:off + sz], in0=y[:, :sz],
                                                   scalar=0.0, in1=en[:, :sz],
                                                   op0=ALU.max, op1=ALU.add)

            # ---- per layer ----
            for l in (1, 2, 3):
                P = LAYERS[l]
                Cout, H, tw = P["Cout"], P["H"], P["tw"]
                xcur = x1T if l == 1 else xbig[l]
                # dense + table build
                with tc.tile_pool(name=f"dense{l}", bufs=2) as dpool:
                    for off, sz in chunks:
                        ph = ps2.tile([Cout, 512], f32, tag="mm")
                        nc.tensor.matmul(out=ph[:, :sz], lhsT=W[P["W"]][:],
                                         rhs=xcur[:, off:off + sz], start=True, stop=True)
                        hTc = dpool.tile([Cout, 512], f32, tag="hTc")
                        nc.vector.tensor_scalar(out=hTc[:, :sz], in0=ph[:, :sz],
                                                scalar1=W[P["C"]][:], scalar2=None,
                                                op0=ALU.subtract)
                        pas = ps2.tile([H, 512], f32, tag="aux")
                        nc.tensor.matmul(out=pas[:, :sz], lhsT=W[P["AS"]][:],
                                         rhs=hTc[:, :sz], start=True, stop=True)
                        asr = dpool.tile([H, 512], f32, tag="asr")
                        nc.vector.tensor_copy(out=asr[:, :sz], in_=pas[:, :sz])
                        pad_ = ps2.tile([H, 512], f32, tag="aux")
                        nc.tensor.matmul(out=pad_[:, :sz], lhsT=W[P["AD"]][:],
                                         rhs=hTc[:, :sz], start=True, stop=True)
                        adr = dpool.tile([H, 512], f32, tag="adr")
                        nc.vector.tensor_copy(out=adr[:, :sz], in_=pad_[:, :sz])
                        for nt in range(sz // 128):
                            g = off // 128 + nt  # global node tile == block id
                            lo = nt * 128
                            pt = ps2.tile([128, Cout], f32, tag="aux")
                            nc.tensor.transpose(out=pt[:], in_=hTc[:, lo:lo + 128],
                                                identity=W["ident"][:Cout, :Cout])
                            stg = dpool.tile([128, tw], bf16, tag="stg")
                            nc.vector.tensor_copy(out=stg[:, 0:Cout], in_=pt[:])
                            pt2 = ps2.tile([128, H], f32, tag="aux")
                            nc.tensor.transpose(out=pt2[:], in_=asr[:, lo:lo + 128],
                                                identity=W["ident"][:H, :H])
                            nc.vector.tensor_copy(out=stg[:, Cout:Cout + H], in_=pt2[:])
                            pt3 = ps2.tile([128, H], f32, tag="aux")
                            nc.tensor.transpose(out=pt3[:], in_=adr[:, lo:lo + 128],
                                                identity=W["ident"][:H, :H])
                            nc.vector.tensor_copy(out=adst_nm[:, g * 4:g * 4 + H], in_=pt3[:])
                            nc.sync.dma_start(
                                out=slices[l].ap()[g * 128:(g + 1) * 128, 0:Cout + H],
                                in_=stg[:, 0:Cout + H])
                # allgather the table (one collective per layer)
                if single:
                    # timing stand-in: move the same bytes via plain DMA
                    for r in range(NCORES):
                        nc.sync.dma_start(
                            out=tabs[l].ap()[r * NLOCP:(r + 1) * NLOCP, :],
                            in_=slices[l].ap()[:, :])
                else:
                    nc.gpsimd.collective_compute(
                        "AllGather", ALU.bypass,
                        replica_groups=[list(range(NCORES))],
                        ins=[slices[l].ap()[:, :]],
                        outs=[tabs[l].ap()],
                    )

                # ---- edge phase ----
                ac = Cout          # a_src column base in gathered rows
                ec = Cout + H      # ex column base
                AW = Cout + 2 * H  # aggregated width
                for b in range(NBLK):
                    Z = wpool.tile([128, NT, tw], bf16, tag="Z")
                    HROWS = (NCORES // 2) * NLOCP
                    for h in range(2):
                        nc.gpsimd.dma_gather(
                            out_ap=Z[:, h * CHT:(h + 1) * CHT, :],
                            in_ap=tabs[l].ap()[h * HROWS:(h + 1) * HROWS, :],
                            idxs_ap=idx_sb[:, (b * 2 + h) * IC:(b * 2 + h + 1) * IC],
                            num_idxs=CH, num_idxs_reg=CH, elem_size=tw,
                            single_packet=False)
                    selA = wpool.tile([128, NT, 128], bf16, tag="selA")
                    nc.vector.tensor_tensor(
                        out=selA[:],
                        in0=dstloc_sb[:, b * NT:(b + 1) * NT][:, :, None].to_broadcast([128, NT, 128]),
                        in1=W["iota_row"][:, None, :].to_broadcast([128, NT, 128]),
                        op=ALU.is_equal)
                    selB = wpool.tile([128, EBLK], bf16, tag="selB")
                    for c5 in range(EBLK // 512):
                        dfp0 = wpool.tile([1, 512], f32, tag="dfp0")
                        nc.sync.dma_start(
                            out=dfp0[:],
                            in_=din["dstflat"].ap()[b:b + 1, c5 * 512:(c5 + 1) * 512])
                        pr = ps2.tile([128, 512], f32, tag="mm")
                        nc.tensor.matmul(out=pr[:], lhsT=W["ones_row"][:],
                                         rhs=dfp0[:], start=True, stop=True)
                        nc.vector.tensor_tensor(
                            out=selB[:, c5 * 512:(c5 + 1) * 512],
                            in0=W["iota_col"][:].to_broadcast([128, 512]),
                            in1=pr[:], op=ALU.is_equal)
                    pad_ps = ps1.tile([128, NT, H], f32, tag="adst")
                    for k in range(NT):
                        nc.tensor.matmul(out=pad_ps[:, k, :],
                                         lhsT=selB[:, k * 128:(k + 1) * 128],
                                         rhs=adst_nm[:, b * 4:b * 4 + H],
                                         start=True, stop=True)
                    s_e = wpool.tile([128, NT, H], f32, tag="se")
                    nc.vector.tensor_tensor(out=s_e[:], in0=Z[:, :, ac:ac + H],
                                            in1=pad_ps[:], op=ALU.add)
                    nc.vector.scalar_tensor_tensor(out=s_e[:], in0=s_e[:],
                                                   scalar=NEG_SLOPE, in1=s_e[:],
                                                   op0=ALU.mult, op1=ALU.max)
                    nc.scalar.activation(out=Z[:, :, ec:ec + H], in_=s_e[:], func=AF.Exp)
                    zv = Z[:, :, 0:Cout].rearrange("p k (h c) -> p k h c", h=H)
                    exb = Z[:, :, ec:ec + H][:, :, :, None].to_broadcast(
                        [128, NT, H, Cout // H])
                    nc.vector.tensor_tensor(out=zv, in0=zv, in1=exb, op=ALU.mult)
                    pagg = ps2.tile([128, AW], f32, tag="agg")
                    for k in range(NT):
                        nc.tensor.matmul(out=pagg[:], lhsT=selA[:, k, :],
                                         rhs=Z[:, k, 0:AW],
                                         start=(k == 0), stop=(k == NT - 1))
                    r4 = wpool.tile([128, H], f32, tag="r4")
                    nc.vector.tensor_scalar(out=r4[:], in0=pagg[:, AW - H:AW],
                                            scalar1=1e-16, scalar2=None, op0=ALU.add)
                    nc.vector.reciprocal(out=r4[:], in_=r4[:])
                    o_sb = wpool.tile([128, Cout], f32, tag="osb")
                    nc.vector.tensor_tensor(
                        out=o_sb[:].rearrange("p (h c) -> p h c", h=H),
                        in0=pagg[:, 0:Cout].rearrange("p (h c) -> p h c", h=H),
                        in1=r4[:][:, :, None].to_broadcast([128, H, Cout // H]),
                        op=ALU.mult)
                    if l < 3:
                        pt = ps2.tile([128, 128], f32, tag="aux")
                        nc.tensor.transpose(out=pt[:], in_=o_sb[:], identity=W["ident"][:])
                        y = wpool.tile([128, 128], f32, tag="yh")
                        nc.vector.tensor_scalar(out=y[:], in0=pt[:],
                                                scalar1=W[f"bn{l}_s"][:],
                                                scalar2=W[f"bn{l}_b"][:],
                                                op0=ALU.mult, op1=ALU.add)
                        ng = wpool.tile([128, 128], f32, tag="ngh")
                        nc.vector.tensor_scalar(out=ng[:], in0=y[:], scalar1=0.0,
                                                scalar2=None, op0=ALU.min)
                        en = wpool.tile([128, 128], f32, tag="enh")
                        nc.scalar.activation(out=en[:], in_=ng[:], func=AF.Exp)
                        nc.vector.scalar_tensor_tensor(
                            out=xbig[l + 1][:, b * 128:(b + 1) * 128], in0=y[:],
                            scalar=0.0, in1=en[:], op0=ALU.max, op1=ALU.add)
                    else:
                        nc.vector.tensor_tensor(out=o_sb[:], in0=o_sb[:],
                                                in1=W["bias3_rep"][:], op=ALU.add)
                        mx = wpool.tile([128, 1], f32, tag="mx")
                        nc.vector.tensor_reduce(out=mx[:], in_=o_sb[:],
                                                axis=mybir.AxisListType.X, op=ALU.max)
                        sh = wpool.tile([128, NCLS], f32, tag="sh")
                        nc.vector.tensor_scalar(out=sh[:], in0=o_sb[:], scalar1=mx[:],
                                                scalar2=None, op0=ALU.subtract)
                        ex8 = wpool.tile([128, NCLS], f32, tag="ex8")
                        se = wpool.tile([128, 1], f32, tag="sse")
                        nc.scalar.activation(out=ex8[:], in_=sh[:], func=AF.Exp,
                                             accum_out=se[:])
                        lnse = wpool.tile([128, 1], f32, tag="lnse")
                        nc.scalar.activation(out=lnse[:], in_=se[:], func=AF.Ln)
                        res = wpool.tile([128, NCLS], f32, tag="res")
                        nc.vector.tensor_scalar(out=res[:], in0=sh[:], scalar1=lnse[:],
                                                scalar2=None, op0=ALU.subtract)
                        nc.sync.dma_start(out=out_dram.ap()[b * 128:(b + 1) * 128, :],
                                          in_=res[:])

    nc.compile()
    return nc


def kernel(x, edge_index, params):
    x = np.asarray(x, np.float32)
    w = _fold_params(params)
    per_core, CH = _preprocess_edges(edge_index)

    nc = _build_nc(CH)

    in_maps = []
    for r in range(NCORES):
        m = dict(w)
        xr = np.zeros((NLOCP, F_IN), np.float32)
        xr[:NLOC] = x[r * NLOC:(r + 1) * NLOC]
        m["xT"] = np.ascontiguousarray(xr.T)
        m["idx16"] = per_core[r]["idx16"]
        m["dstloc"] = per_core[r]["dstloc"]
        m["dstflat"] = per_core[r]["dstflat"]
        in_maps.append(m)

    from concourse.bass_utils import run_bass_kernel_spmd
    import os
    trace = os.environ.get("GAT_TRACE", "0") == "1"
    res = run_bass_kernel_spmd(nc, in_maps, core_ids=list(range(NCORES)), trace=trace)
    if trace:
        kernel.last_results = res
        print("HW exec time:", res.exec_time_ns, "ns")
    out = np.concatenate([res.results[r]["out"][:NLOC] for r in range(NCORES)], axis=0)
    return out
